# revision 17
# baseline (speedup 1.0000x reference)
"""Trainium2 Bass kernel for Conformer-style relative-position MHSA.

Sharding: data-parallel over batch — B=8 batch elements, one per NeuronCore.

Hybrid design: baseline's attention pipeline (fp16 shift round trip, bf16
score matmuls, bf16 transposes, scalar exp from bf16 PSUM) + fp8e4
DoubleRow matmuls (2x PE throughput, measured ~377ns/op in-context) for
the K>=256 GEMMs: QKVP projections, AV, and the output projection.
Weights are prescaled x64 on host for fp8 range; the 4096x score scale is
folded into the exp() scale and the final eviction scale. Softmax
denominator via a ones column in the AV stationary (M=128 padded),
normalized through a gpsimd partition_broadcast + fast reciprocal.
"""

import sys

for _p in ("/opt/trn_rl_repo", "/root/.axon_site/_ro/pypackages"):
    if _p not in sys.path:
        sys.path.insert(0, _p)

import numpy as np
import ml_dtypes

import concourse.bass as bass
import concourse.mybir as mybir
import concourse.tile as tile
from concourse import bacc
from concourse.bass_utils import run_bass_kernel_spmd
from concourse.masks import make_identity

F32 = mybir.dt.float32
BF16 = mybir.dt.bfloat16
FP16 = mybir.dt.float16
FP8 = mybir.dt.float8e4
DR = mybir.MatmulPerfMode.DoubleRow
AX = mybir.AluOpType
AF = mybir.ActivationFunctionType

P = 128
T = 1024
D = 512
H = 8
O = 64
KT = D // P
NT = T // P
NCH = T // 512
LN_EPS = 1e-3
SC = 64.0
INV_SCORE = 1.0 / 4096.0


def build_nc():
    nc = bacc.Bacc("TRN2", target_bir_lowering=False)

    x_res = nc.dram_tensor("x_res", [P, NT, D], F32, kind="ExternalInput")
    post = nc.dram_tensor("post", [P, KT, T], FP8, kind="ExternalInput")
    wq = nc.dram_tensor("wq", [P, KT, D], FP8, kind="ExternalInput")
    wk = nc.dram_tensor("wk", [P, KT, D], FP8, kind="ExternalInput")
    wv = nc.dram_tensor("wv", [P, KT, D], FP8, kind="ExternalInput")
    wp = nc.dram_tensor("wp", [P, KT, D], FP8, kind="ExternalInput")
    wo = nc.dram_tensor("wo", [P, KT, D], FP8, kind="ExternalInput")
    u_in = nc.dram_tensor("u_in", [P, KT], F32, kind="ExternalInput")
    v_in = nc.dram_tensor("v_in", [P, KT], F32, kind="ExternalInput")
    beta_in = nc.dram_tensor("beta_in", [P, D], BF16, kind="ExternalInput")
    out = nc.dram_tensor("out", [T, D], F32, kind="ExternalOutput")

    with tile.TileContext(nc) as tc:
        with (
            tc.tile_pool(name="consts", bufs=1) as consts,
            tc.tile_pool(name="acts", bufs=1) as acts,
            tc.tile_pool(name="dram", bufs=2, space="DRAM") as dram_pool,
        ):
            xres_sb = acts.tile([P, NT, D], F32)
            nc.sync.dma_start(xres_sb[:], x_res[:])
            beta_sb = consts.tile([P, D], BF16, tag="beta")
            nc.sync.dma_start(beta_sb[:], beta_in[:])
            eps_sb = consts.tile([P, 1], F32, tag="eps")
            nc.vector.memset(eps_sb[:], LN_EPS)
            wu = consts.tile([P, 2, 512], FP8, tag="wu")
            nc.vector.memset(wu[:], 0.125)
            ident8 = consts.tile([P, P], FP8)
            make_identity(nc, ident8)
            identb = consts.tile([P, P], BF16)
            make_identity(nc, identb)
            ones_bc = consts.tile([P, O], BF16, tag="ones_bc")
            nc.vector.memset(ones_bc[:], 1.0)

            xlnT = acts.tile([P, KT, T], FP8)
            qu = acts.tile([P, KT, T], BF16)
            qv = acts.tile([P, KT, T], BF16)
            kT_sb = acts.tile([P, KT, T], BF16)
            pT_sb = acts.tile([P, KT, T], BF16)
            outT = acts.tile([P, KT, T], FP8)
            avw = acts.tile([P, NT, H, P], FP8)
            nc.gpsimd.memset(avw[:, :, :, O + 1:], 0.0)
            nc.gpsimd.memset(avw[:, :, :, O:O + 1], 1.0)

            with (
                tc.tile_pool(name="ln_tmp", bufs=4) as ln_tmp,
                tc.tile_pool(name="psB", bufs=2, space="PSUM") as psB,
                tc.tile_pool(name="psP", bufs=6, space="PSUM") as psP,
            ):
                # warm-up matmuls: ramp PE clock while DMA + LN run
                with nc.named_scope("warmup"):
                    for r in range(30):
                        ps_w = psP.tile([P, 512], F32, tag="ps", name="ps")
                        nc.tensor.matmul(
                            ps_w[:], wu[:, :, 0:P], wu[:],
                            start=True, stop=True, perf_mode=DR)

                xln_nd = ln_tmp.tile([P, NT, D], FP8, tag="xln_nd")
                with nc.named_scope("ln"):
                    for nt in range(NT):
                        st6 = ln_tmp.tile([P, 6], F32, tag="st6")
                        nc.vector.bn_stats(out=st6[:], in_=xres_sb[:, nt, :])
                        mv = ln_tmp.tile([P, 2], F32, tag="mv")
                        nc.vector.bn_aggr(out=mv[:], in_=st6[:])
                        sd = ln_tmp.tile([P, 1], F32, tag="sd")
                        nc.scalar.activation(out=sd[:], in_=mv[:, 1:2],
                                             func=AF.Sqrt, bias=eps_sb[:])
                        rstd = ln_tmp.tile([P, 1], F32, tag="rstd")
                        nc.vector.reciprocal(rstd[:], sd[:])
                        xt = ln_tmp.tile([P, D], BF16, tag="xt")
                        nc.vector.tensor_scalar(
                            out=xt[:], in0=xres_sb[:, nt, :],
                            scalar1=mv[:, 0:1], scalar2=rstd[:],
                            op0=AX.subtract, op1=AX.mult)
                        nc.vector.tensor_tensor(
                            xln_nd[:, nt, :], xt[:], beta_sb[:], op=AX.add)
                    for kt in range(KT):
                        ps_x = psB.tile([P, T, 2], FP8, tag="tx")
                        for nt in range(NT):
                            nc.tensor.transpose(
                                ps_x[:, bass.ts(nt, P), 0],
                                xln_nd[:, nt, bass.ts(kt, P)],
                                ident8[:])
                        nc.scalar.copy(xlnT[:, kt, :], ps_x[:, :, 0])

                post_sb = acts.tile([P, KT, T], FP8)
                nc.sync.dma_start(post_sb[:], post[:])
                w_sb = {}
                for name, t in (("wq", wq), ("wk", wk), ("wv", wv),
                                ("wp", wp), ("wo", wo)):
                    w_sb[name] = consts.tile([P, KT, D], FP8, tag=f"w_{name}",
                                             name=f"w_{name}")
                    nc.sync.dma_start(w_sb[name][:], t[:])
                u_sb = consts.tile([P, KT], F32, tag="u")
                nc.sync.dma_start(u_sb[:], u_in[:])
                v_sb = consts.tile([P, KT], F32, tag="v")
                nc.sync.dma_start(v_sb[:], v_in[:])

                # ---- projections: fp8 DoubleRow over kt pairs ----
                with nc.named_scope("proj"):
                    for mch in range(KT):
                        for nch in range(NCH):
                            ps_q = psP.tile([P, 512], F32, tag="ps", name="ps")
                            for j in range(2):
                                nc.tensor.matmul(
                                    ps_q[:],
                                    w_sb["wq"][:, 2 * j:2 * j + 2,
                                               bass.ts(mch, P)],
                                    xlnT[:, 2 * j:2 * j + 2,
                                         bass.ts(nch, 512)],
                                    start=(j == 0), stop=(j == 1),
                                    perf_mode=DR)
                            nc.scalar.add(qu[:, mch, bass.ts(nch, 512)],
                                          ps_q[:], u_sb[:, mch:mch + 1])
                            nc.vector.tensor_scalar(
                                out=qv[:, mch, bass.ts(nch, 512)],
                                in0=ps_q[:], scalar1=v_sb[:, mch:mch + 1],
                                scalar2=None, op0=AX.add)
                        for nch in range(NCH):
                            ps_k = psP.tile([P, 512], F32, tag="ps", name="ps")
                            for j in range(2):
                                nc.tensor.matmul(
                                    ps_k[:],
                                    w_sb["wk"][:, 2 * j:2 * j + 2,
                                               bass.ts(mch, P)],
                                    xlnT[:, 2 * j:2 * j + 2,
                                         bass.ts(nch, 512)],
                                    start=(j == 0), stop=(j == 1),
                                    perf_mode=DR)
                            nc.vector.tensor_copy(
                                kT_sb[:, mch, bass.ts(nch, 512)], ps_k[:])
                        for nch in range(NCH):
                            ps_p = psP.tile([P, 512], F32, tag="ps", name="ps")
                            for j in range(2):
                                nc.tensor.matmul(
                                    ps_p[:],
                                    w_sb["wp"][:, 2 * j:2 * j + 2,
                                               bass.ts(mch, P)],
                                    post_sb[:, 2 * j:2 * j + 2,
                                            bass.ts(nch, 512)],
                                    start=(j == 0), stop=(j == 1),
                                    perf_mode=DR)
                            nc.vector.tensor_copy(
                                pT_sb[:, mch, bass.ts(nch, 512)], ps_p[:])
                    for mt in range(NT):
                        ps_v = psP.tile([P, 512], F32, tag="ps", name="ps")
                        for j in range(2):
                            nc.tensor.matmul(
                                ps_v[:],
                                xlnT[:, 2 * j:2 * j + 2, bass.ts(mt, P)],
                                w_sb["wv"][:, 2 * j:2 * j + 2, :],
                                start=(j == 0), stop=(j == 1),
                                perf_mode=DR)
                        nc.vector.tensor_copy(
                            avw[:, mt, :, 0:O],
                            ps_v[:].rearrange("p (h o) -> p h o", h=H))

            # ====== attention: software-pipelined across head pairs ======
            with (
                tc.tile_pool(name="ywr", bufs=4) as ywr_pool,
                tc.tile_pool(name="bds", bufs=5) as bds_pool,
                tc.tile_pool(name="sfull", bufs=1) as s_pool,
                tc.tile_pool(name="et", bufs=1) as et_pool,
                tc.tile_pool(name="avsb", bufs=2) as avsb_pool,
                tc.tile_pool(name="ps_bd", bufs=2, space="PSUM") as ps_bd_pool,
                tc.tile_pool(name="ps_s", bufs=3, space="PSUM") as ps_s_pool,
                tc.tile_pool(name="ps_av", bufs=2, space="PSUM") as ps_av_pool,
                tc.tile_pool(name="psT", bufs=1, space="PSUM") as psT,
            ):
                NPAIR = H // 2
                ydram_all = {}
                s_all = {}
                et_all = {}

                def emit_bd_nt(pair, nt):
                    heads = (2 * pair, 2 * pair + 1)
                    ywr = {}
                    for h in heads:
                        ywr[h] = ywr_pool.tile(
                            [P, T + 1], FP16,
                            tag=f"ywr{h % 2}", name=f"ywr{h % 2}")
                        nc.gpsimd.memset(ywr[h][:, 0:1], 0.0)
                    for h in heads:
                        base = (h % 2) * O
                        ps_bd = [ps_bd_pool.tile([P, 512], F32, tag="ps",
                                                 name="ps")
                                 for _ in range(NCH)]
                        for mch in range(NCH):
                            nc.tensor.matmul(
                                ps_bd[mch][:],
                                qv[base:base + O, pair, bass.ts(nt, P)],
                                pT_sb[base:base + O, pair, bass.ts(mch, 512)],
                                start=True, stop=True)
                        nc.vector.tensor_copy(ywr[h][:, 1:513], ps_bd[0][:])
                        nc.scalar.copy(ywr[h][:, 513:1025], ps_bd[1][:])
                    for h in heads:
                        nc.gpsimd.dma_start(
                            ydram_all[pair][h][bass.ts(nt, P), :], ywr[h][:])

                def emit_acs_nt(pair, nt):
                    heads = (2 * pair, 2 * pair + 1)
                    bds = {}
                    for h in heads:
                        bds[h] = bds_pool.tile(
                            [P, T], FP16, tag=f"bds{h % 2}",
                            name=f"bds{h % 2}")
                        yflat = ydram_all[pair][h].flatten()
                        start = T * (nt * P + 1)
                        nc.sync.dma_start(
                            bds[h][:],
                            yflat[start:start + P * T].rearrange(
                                "(a b) -> a b", b=T))
                    for h in heads:
                        base = (h % 2) * O
                        ps_s = [ps_s_pool.tile([P, 512], F32, tag="ps",
                                               name="ps")
                                for _ in range(NCH)]
                        for mch in range(NCH):
                            nc.tensor.matmul(
                                ps_s[mch][:],
                                qu[base:base + O, pair, bass.ts(nt, P)],
                                kT_sb[base:base + O, pair, bass.ts(mch, 512)],
                                start=True, stop=True)
                        for mch in range(NCH):
                            nc.vector.tensor_tensor(
                                out=s_all[pair][h][:, nt, bass.ts(mch, 512)],
                                in0=ps_s[mch][:],
                                in1=bds[h][:, bass.ts(mch, 512)],
                                op=AX.add)

                def emit_tx(pair, h, mt):
                    ps_t = psT.tile([P, T], BF16, tag="tx", name="ps_t")
                    for nt in range(NT):
                        nc.tensor.transpose(
                            ps_t[:, bass.ts(nt, P)],
                            s_all[pair][h][:, nt, bass.ts(mt, P)],
                            identb[:])
                    nc.scalar.activation(
                        out=et_all[pair][h][:, mt, :], in_=ps_t[:],
                        func=AF.Exp, scale=INV_SCORE)

                av_ps = {}

                def emit_av_mt(pair, h, jm):
                    if (pair, h) not in av_ps:
                        av_ps[(pair, h)] = [
                            ps_av_pool.tile([P, 512], F32, tag="ps", name="ps")
                            for _ in range(NCH)]
                    ps_av = av_ps[(pair, h)]
                    et = et_all[pair]
                    for nch in range(NCH):
                        nc.tensor.matmul(
                            ps_av[nch][:],
                            avw[:, 2 * jm:2 * jm + 2, h, :],
                            et[h][:, 2 * jm:2 * jm + 2, bass.ts(nch, 512)],
                            start=(jm == 0), stop=(jm == NT // 2 - 1),
                            perf_mode=DR)

                def emit_av_fin(pair, h):
                    base = (h % 2) * O
                    kt = h // 2
                    ps_av = av_ps.pop((pair, h))
                    for nch in range(NCH):
                        av_sb = avsb_pool.tile([O + 1, 512], BF16,
                                               tag=f"avsb{h % 2}",
                                               name=f"avsb{h % 2}")
                        nc.scalar.copy(av_sb[:], ps_av[nch][0:O + 1, :])
                        ps_bc = ps_bd_pool.tile([P, 512], F32, tag="ps",
                                                name="ps")
                        nc.tensor.matmul(
                            ps_bc[0:O, :],
                            ones_bc[O:O + 1, :],
                            av_sb[O:O + 1, :],
                            start=True, stop=True)
                        rb = avsb_pool.tile([O, 512], F32, tag=f"rb{h % 2}",
                                            name=f"rb{h % 2}")
                        nc.vector.reciprocal_approx_fast(
                            out=rb[:], in_=ps_bc[0:O, :])
                        nc.gpsimd.tensor_tensor(
                            out=outT[base:base + O, kt, bass.ts(nch, 512)],
                            in0=av_sb[0:O, :], in1=rb[:], op=AX.mult)

                for p in range(NPAIR + 1):
                    if p < NPAIR:
                        heads = (2 * p, 2 * p + 1)
                        ydram_all[p] = {
                            h: dram_pool.tile([T, T + 1], FP16,
                                              tag=f"y{h % 2}",
                                              name=f"y{h % 2}")
                            for h in heads}
                        s_all[p] = {
                            h: s_pool.tile([P, NT, T], BF16,
                                           tag=f"s{h % 2}", name=f"s{h % 2}")
                            for h in heads}
                        et_all[p] = {
                            h: et_pool.tile([P, NT, T], FP8,
                                            tag=f"et{h % 2}",
                                            name=f"et{h % 2}")
                            for h in heads}
                    for i in range(NT):
                        if 1 <= p <= NPAIR:
                            emit_acs_nt(p - 1, i)
                        if p < NPAIR:
                            emit_bd_nt(p, i)
                    if 1 <= p <= NPAIR:
                        with nc.named_scope("tx"):
                            for h in (2 * (p - 1), 2 * (p - 1) + 1):
                                for mt in range(NT):
                                    emit_tx(p - 1, h, mt)
                        with nc.named_scope("avf"):
                            for h in (2 * (p - 1), 2 * (p - 1) + 1):
                                for jm in range(NT // 2):
                                    emit_av_mt(p - 1, h, jm)
                                emit_av_fin(p - 1, h)

            # ---- output projection + residual ----
            with (
                tc.tile_pool(name="fin", bufs=4) as fin_pool,
                tc.tile_pool(name="ps_y", bufs=4, space="PSUM") as ps_y_pool,
            ):
                with nc.named_scope("out"):
                    for nt in range(NT):
                        ps_y = ps_y_pool.tile([P, D], F32, tag="ps",
                                              name="ps")
                        for j in range(2):
                            nc.tensor.matmul(
                                ps_y[:],
                                outT[:, 2 * j:2 * j + 2, bass.ts(nt, P)],
                                w_sb["wo"][:, 2 * j:2 * j + 2, :],
                                start=(j == 0), stop=(j == 1),
                                perf_mode=DR)
                        fin = fin_pool.tile([P, D], F32)
                        nc.vector.scalar_tensor_tensor(
                            out=fin[:], in0=ps_y[:], scalar=INV_SCORE,
                            in1=xres_sb[:, nt, :], op0=AX.mult, op1=AX.add)
                        nc.sync.dma_start(out[bass.ts(nt, P), :], fin[:])

    nc.compile()
    return nc


_NC = None


def _get_nc():
    global _NC
    if _NC is None:
        _NC = build_nc()
    return _NC


def _run(inputs_dict, trace=False, trace_cores=None):
    f8 = ml_dtypes.float8_e4m3
    bf = ml_dtypes.bfloat16
    inputs = np.asarray(inputs_dict["inputs"], np.float32)
    pos = np.asarray(inputs_dict["pos"], np.float32)
    gamma = np.asarray(inputs_dict["gamma"], np.float32)
    beta = np.asarray(inputs_dict["beta"], np.float32)
    qk = np.asarray(inputs_dict["query_kernel"], np.float32)
    kk = np.asarray(inputs_dict["key_kernel"], np.float32)
    vk = np.asarray(inputs_dict["value_kernel"], np.float32)
    pk = np.asarray(inputs_dict["pos_kernel"], np.float32)
    u = np.asarray(inputs_dict["pos_bias_u"], np.float32)
    v = np.asarray(inputs_dict["pos_bias_v"], np.float32)
    prk = np.asarray(inputs_dict["projection_kernel"], np.float32)
    pbias = np.asarray(inputs_dict["projection_bias"], np.float32)

    scale = 1.0 / np.sqrt(np.float32(O))

    def wcat(w, rowscale=None):
        c = np.transpose(w, (1, 0, 2)).reshape(D, H * O)
        if rowscale is not None:
            c = c * rowscale[:, None]
        return np.ascontiguousarray(
            c.reshape(KT, P, H * O).transpose(1, 0, 2)).astype(f8)

    wq_c = wcat(qk * SC, gamma)
    wk_c = wcat(kk * SC * scale, gamma)
    wv_c = wcat(vk * SC, gamma)
    wp_c = wcat(pk * SC * scale)
    wo_c = np.ascontiguousarray(
        (prk * SC).reshape(H * O, D).reshape(KT, P, D)
        .transpose(1, 0, 2)).astype(f8)
    u_c = np.ascontiguousarray(
        (u * SC).reshape(H * O).reshape(KT, P).T).astype(np.float32)
    v_c = np.ascontiguousarray(
        (v * SC).reshape(H * O).reshape(KT, P).T).astype(np.float32)
    beta_adj = np.where(gamma != 0, beta / np.where(gamma == 0, 1, gamma), 0.0)
    beta_b = np.broadcast_to(beta_adj[None, :], (P, D)).astype(bf).copy()

    in_maps = []
    for b in range(8):
        x_b = inputs[b]
        in_maps.append({
            "x_res": np.ascontiguousarray(
                x_b.reshape(NT, P, D).transpose(1, 0, 2)).astype(np.float32),
            "post": np.ascontiguousarray(
                pos[b].T.reshape(KT, P, T).transpose(1, 0, 2)).astype(f8),
            "wq": wq_c, "wk": wk_c, "wv": wv_c, "wp": wp_c, "wo": wo_c,
            "u_in": u_c, "v_in": v_c, "beta_in": beta_b,
        })

    nc = _get_nc()
    res = run_bass_kernel_spmd(
        nc, in_maps, core_ids=list(range(8)), trace=trace,
        trace_cores=trace_cores)
    outs = np.stack([np.asarray(r["out"], np.float32) for r in res.results])
    outs = outs + pbias[None, None, :]
    return outs, res


def kernel(**inputs):
    outs, _ = _run(inputs)
    return outs


if __name__ == "__main__":
    nc = build_nc()
    print("built ok")


# revision 18
# speedup vs baseline: 1.0819x; 1.0819x over previous
"""Trainium2 Bass kernel for Conformer-style relative-position MHSA.

Sharding: data-parallel over batch — B=8 batch elements, one per NeuronCore.

Hybrid design: baseline's attention pipeline (fp16 shift round trip, bf16
score matmuls, bf16 transposes, scalar exp from bf16 PSUM) + fp8e4
DoubleRow matmuls (2x PE throughput, measured ~377ns/op in-context) for
the K>=256 GEMMs: QKVP projections, AV, and the output projection.
Weights are prescaled x64 on host for fp8 range; the 4096x score scale is
folded into the exp() scale and the final eviction scale. Softmax
denominator via a ones column in the AV stationary (M=128 padded),
normalized through a gpsimd partition_broadcast + fast reciprocal.
"""

import sys

for _p in ("/opt/trn_rl_repo", "/root/.axon_site/_ro/pypackages"):
    if _p not in sys.path:
        sys.path.insert(0, _p)

import numpy as np
import ml_dtypes

import concourse.bass as bass
import concourse.mybir as mybir
import concourse.tile as tile
from concourse import bacc
from concourse.bass_utils import run_bass_kernel_spmd
from concourse.masks import make_identity

F32 = mybir.dt.float32
BF16 = mybir.dt.bfloat16
FP16 = mybir.dt.float16
FP8 = mybir.dt.float8e4
DR = mybir.MatmulPerfMode.DoubleRow
AX = mybir.AluOpType
AF = mybir.ActivationFunctionType

P = 128
T = 1024
D = 512
H = 8
O = 64
KT = D // P
NT = T // P
NCH = T // 512
LN_EPS = 1e-3
SC = 64.0
INV_SCORE = 1.0 / 4096.0


def build_nc():
    nc = bacc.Bacc("TRN2", target_bir_lowering=False)

    x_res = nc.dram_tensor("x_res", [P, NT, D], F32, kind="ExternalInput")
    post = nc.dram_tensor("post", [P, KT, T], FP8, kind="ExternalInput")
    wq = nc.dram_tensor("wq", [P, KT, D], FP8, kind="ExternalInput")
    wk = nc.dram_tensor("wk", [P, KT, D], FP8, kind="ExternalInput")
    wv = nc.dram_tensor("wv", [P, KT, D], FP8, kind="ExternalInput")
    wp = nc.dram_tensor("wp", [P, KT, D], FP8, kind="ExternalInput")
    wo = nc.dram_tensor("wo", [P, KT, D], FP8, kind="ExternalInput")
    u_in = nc.dram_tensor("u_in", [P, KT], F32, kind="ExternalInput")
    v_in = nc.dram_tensor("v_in", [P, KT], F32, kind="ExternalInput")
    beta_in = nc.dram_tensor("beta_in", [P, D], BF16, kind="ExternalInput")
    out = nc.dram_tensor("out", [T, D], F32, kind="ExternalOutput")

    with tile.TileContext(nc) as tc:
        with (
            tc.tile_pool(name="consts", bufs=1) as consts,
            tc.tile_pool(name="acts", bufs=1) as acts,
            tc.tile_pool(name="dram", bufs=2, space="DRAM") as dram_pool,
        ):
            xres_sb = acts.tile([P, NT, D], F32)
            nc.sync.dma_start(xres_sb[:], x_res[:])
            beta_sb = consts.tile([P, D], BF16, tag="beta")
            nc.sync.dma_start(beta_sb[:], beta_in[:])
            eps_sb = consts.tile([P, 1], F32, tag="eps")
            nc.vector.memset(eps_sb[:], LN_EPS)
            ident8 = consts.tile([P, P], FP8)
            make_identity(nc, ident8)
            identb = consts.tile([P, P], BF16)
            make_identity(nc, identb)
            ones_bc = consts.tile([P, O], BF16, tag="ones_bc")
            nc.vector.memset(ones_bc[:], 1.0)

            xlnT = acts.tile([P, KT, T], FP8)
            qu = acts.tile([P, KT, T], BF16)
            qv = acts.tile([P, KT, T], BF16)
            kT_sb = acts.tile([P, KT, T], BF16)
            pT_sb = acts.tile([P, KT, T], BF16)
            outT = acts.tile([P, KT, T], FP8)
            avw = acts.tile([P, NT, H, P], FP8)
            nc.gpsimd.memset(avw[:, :, :, O + 1:], 0.0)
            nc.gpsimd.memset(avw[:, :, :, O:O + 1], 1.0)

            with (
                tc.tile_pool(name="ln_tmp", bufs=4) as ln_tmp,
                tc.tile_pool(name="psB", bufs=2, space="PSUM") as psB,
                tc.tile_pool(name="psP", bufs=6, space="PSUM") as psP,
            ):
                # warm-up matmuls: ramp PE clock while DMA + LN run
                wu = ln_tmp.tile([P, 2, 512], FP8, tag="wu")
                nc.gpsimd.memset(wu[:], 0.125)
                with nc.named_scope("warmup"):
                    for r in range(30):
                        ps_w = psP.tile([P, 512], F32, tag="ps", name="ps")
                        nc.tensor.matmul(
                            ps_w[:], wu[:, :, 0:P], wu[:],
                            start=True, stop=True, perf_mode=DR)

                xln_nd = ln_tmp.tile([P, NT, D], FP8, tag="xln_nd")
                with nc.named_scope("ln"):
                    for nt in range(NT):
                        st6 = ln_tmp.tile([P, 6], F32, tag="st6")
                        nc.vector.bn_stats(out=st6[:], in_=xres_sb[:, nt, :])
                        mv = ln_tmp.tile([P, 2], F32, tag="mv")
                        nc.vector.bn_aggr(out=mv[:], in_=st6[:])
                        sd = ln_tmp.tile([P, 1], F32, tag="sd")
                        nc.scalar.activation(out=sd[:], in_=mv[:, 1:2],
                                             func=AF.Sqrt, bias=eps_sb[:])
                        rstd = ln_tmp.tile([P, 1], F32, tag="rstd")
                        nc.vector.reciprocal(rstd[:], sd[:])
                        xt = ln_tmp.tile([P, D], BF16, tag="xt")
                        nc.vector.tensor_scalar(
                            out=xt[:], in0=xres_sb[:, nt, :],
                            scalar1=mv[:, 0:1], scalar2=rstd[:],
                            op0=AX.subtract, op1=AX.mult)
                        nc.vector.tensor_tensor(
                            xln_nd[:, nt, :], xt[:], beta_sb[:], op=AX.add)
                    for kt in range(KT):
                        ps_x = psB.tile([P, T, 2], FP8, tag="tx")
                        for nt in range(NT):
                            nc.tensor.transpose(
                                ps_x[:, bass.ts(nt, P), 0],
                                xln_nd[:, nt, bass.ts(kt, P)],
                                ident8[:])
                        nc.scalar.copy(xlnT[:, kt, :], ps_x[:, :, 0])

                post_sb = acts.tile([P, KT, T], FP8)
                nc.sync.dma_start(post_sb[:], post[:])
                w_sb = {}
                for name, t in (("wq", wq), ("wk", wk), ("wv", wv),
                                ("wp", wp), ("wo", wo)):
                    w_sb[name] = consts.tile([P, KT, D], FP8, tag=f"w_{name}",
                                             name=f"w_{name}")
                    nc.sync.dma_start(w_sb[name][:], t[:])
                u_sb = consts.tile([P, KT], F32, tag="u")
                nc.sync.dma_start(u_sb[:], u_in[:])
                v_sb = consts.tile([P, KT], F32, tag="v")
                nc.sync.dma_start(v_sb[:], v_in[:])

                # ---- projections: fp8 DoubleRow over kt pairs ----
                with nc.named_scope("proj"):
                    for mch in range(KT):
                        for nch in range(NCH):
                            ps_q = psP.tile([P, 512], F32, tag="ps", name="ps")
                            for j in range(2):
                                nc.tensor.matmul(
                                    ps_q[:],
                                    w_sb["wq"][:, 2 * j:2 * j + 2,
                                               bass.ts(mch, P)],
                                    xlnT[:, 2 * j:2 * j + 2,
                                         bass.ts(nch, 512)],
                                    start=(j == 0), stop=(j == 1),
                                    perf_mode=DR)
                            nc.scalar.add(qu[:, mch, bass.ts(nch, 512)],
                                          ps_q[:], u_sb[:, mch:mch + 1])
                            nc.vector.tensor_scalar(
                                out=qv[:, mch, bass.ts(nch, 512)],
                                in0=ps_q[:], scalar1=v_sb[:, mch:mch + 1],
                                scalar2=None, op0=AX.add)
                        for nch in range(NCH):
                            ps_k = psP.tile([P, 512], F32, tag="ps", name="ps")
                            for j in range(2):
                                nc.tensor.matmul(
                                    ps_k[:],
                                    w_sb["wk"][:, 2 * j:2 * j + 2,
                                               bass.ts(mch, P)],
                                    xlnT[:, 2 * j:2 * j + 2,
                                         bass.ts(nch, 512)],
                                    start=(j == 0), stop=(j == 1),
                                    perf_mode=DR)
                            nc.vector.tensor_copy(
                                kT_sb[:, mch, bass.ts(nch, 512)], ps_k[:])
                        for nch in range(NCH):
                            ps_p = psP.tile([P, 512], F32, tag="ps", name="ps")
                            for j in range(2):
                                nc.tensor.matmul(
                                    ps_p[:],
                                    w_sb["wp"][:, 2 * j:2 * j + 2,
                                               bass.ts(mch, P)],
                                    post_sb[:, 2 * j:2 * j + 2,
                                            bass.ts(nch, 512)],
                                    start=(j == 0), stop=(j == 1),
                                    perf_mode=DR)
                            nc.vector.tensor_copy(
                                pT_sb[:, mch, bass.ts(nch, 512)], ps_p[:])
                    for mt in range(NT):
                        ps_v = psP.tile([P, 512], F32, tag="ps", name="ps")
                        for j in range(2):
                            nc.tensor.matmul(
                                ps_v[:],
                                xlnT[:, 2 * j:2 * j + 2, bass.ts(mt, P)],
                                w_sb["wv"][:, 2 * j:2 * j + 2, :],
                                start=(j == 0), stop=(j == 1),
                                perf_mode=DR)
                        nc.vector.tensor_copy(
                            avw[:, mt, :, 0:O],
                            ps_v[:].rearrange("p (h o) -> p h o", h=H))

            # ====== attention: software-pipelined across head pairs ======
            with (
                tc.tile_pool(name="ywr", bufs=4) as ywr_pool,
                tc.tile_pool(name="bds", bufs=5) as bds_pool,
                tc.tile_pool(name="sfull", bufs=1) as s_pool,
                tc.tile_pool(name="et", bufs=1) as et_pool,
                tc.tile_pool(name="avsb", bufs=2) as avsb_pool,
                tc.tile_pool(name="ps_bd", bufs=2, space="PSUM") as ps_bd_pool,
                tc.tile_pool(name="ps_s", bufs=2, space="PSUM") as ps_s_pool,
                tc.tile_pool(name="ps_av", bufs=2, space="PSUM") as ps_av_pool,
                tc.tile_pool(name="psT", bufs=2, space="PSUM") as psT,
            ):
                NPAIR = H // 2
                ydram_all = {}
                s_all = {}
                et_all = {}

                def emit_bd_nt(pair, nt):
                    heads = (2 * pair, 2 * pair + 1)
                    ywr = {}
                    for h in heads:
                        ywr[h] = ywr_pool.tile(
                            [P, T + 1], FP16,
                            tag=f"ywr{h % 2}", name=f"ywr{h % 2}")
                        nc.gpsimd.memset(ywr[h][:, 0:1], 0.0)
                    for h in heads:
                        base = (h % 2) * O
                        ps_bd = [ps_bd_pool.tile([P, 512], F32, tag="ps",
                                                 name="ps")
                                 for _ in range(NCH)]
                        for mch in range(NCH):
                            nc.tensor.matmul(
                                ps_bd[mch][:],
                                qv[base:base + O, pair, bass.ts(nt, P)],
                                pT_sb[base:base + O, pair, bass.ts(mch, 512)],
                                start=True, stop=True)
                        nc.vector.tensor_copy(ywr[h][:, 1:513], ps_bd[0][:])
                        nc.scalar.copy(ywr[h][:, 513:1025], ps_bd[1][:])
                    for h in heads:
                        nc.gpsimd.dma_start(
                            ydram_all[pair][h][bass.ts(nt, P), :], ywr[h][:])

                def emit_acs_nt(pair, nt):
                    heads = (2 * pair, 2 * pair + 1)
                    bds = {}
                    for h in heads:
                        bds[h] = bds_pool.tile(
                            [P, T], FP16, tag=f"bds{h % 2}",
                            name=f"bds{h % 2}")
                        yflat = ydram_all[pair][h].flatten()
                        start = T * (nt * P + 1)
                        nc.sync.dma_start(
                            bds[h][:],
                            yflat[start:start + P * T].rearrange(
                                "(a b) -> a b", b=T))
                    for h in heads:
                        base = (h % 2) * O
                        ps_s = [ps_s_pool.tile([P, 512], F32, tag="ps",
                                               name="ps")
                                for _ in range(NCH)]
                        for mch in range(NCH):
                            nc.tensor.matmul(
                                ps_s[mch][:],
                                qu[base:base + O, pair, bass.ts(nt, P)],
                                kT_sb[base:base + O, pair, bass.ts(mch, 512)],
                                start=True, stop=True)
                        for mch in range(NCH):
                            nc.vector.tensor_tensor(
                                out=s_all[pair][h][:, nt, bass.ts(mch, 512)],
                                in0=ps_s[mch][:],
                                in1=bds[h][:, bass.ts(mch, 512)],
                                op=AX.add)

                def emit_tx(pair, h, mt):
                    ps_t = psT.tile([P, T], BF16, tag="tx", name="ps_t")
                    for nt in range(NT):
                        nc.tensor.transpose(
                            ps_t[:, bass.ts(nt, P)],
                            s_all[pair][h][:, nt, bass.ts(mt, P)],
                            identb[:])
                    nc.scalar.activation(
                        out=et_all[pair][h][:, mt, :], in_=ps_t[:],
                        func=AF.Exp, scale=INV_SCORE)

                av_ps = {}

                def emit_av_mt(pair, h, jm):
                    if (pair, h) not in av_ps:
                        av_ps[(pair, h)] = [
                            ps_av_pool.tile([P, 512], F32, tag="ps", name="ps")
                            for _ in range(NCH)]
                    ps_av = av_ps[(pair, h)]
                    et = et_all[pair]
                    for nch in range(NCH):
                        nc.tensor.matmul(
                            ps_av[nch][:],
                            avw[:, 2 * jm:2 * jm + 2, h, :],
                            et[h][:, 2 * jm:2 * jm + 2, bass.ts(nch, 512)],
                            start=(jm == 0), stop=(jm == NT // 2 - 1),
                            perf_mode=DR)

                def emit_av_fin(pair, h):
                    base = (h % 2) * O
                    kt = h // 2
                    ps_av = av_ps.pop((pair, h))
                    for nch in range(NCH):
                        av_sb = avsb_pool.tile([O + 1, 512], BF16,
                                               tag=f"avsb{h % 2}",
                                               name=f"avsb{h % 2}")
                        nc.scalar.copy(av_sb[:], ps_av[nch][0:O + 1, :])
                        ps_bc = ps_bd_pool.tile([P, 512], F32, tag="ps",
                                                name="ps")
                        nc.tensor.matmul(
                            ps_bc[0:O, :],
                            ones_bc[O:O + 1, :],
                            av_sb[O:O + 1, :],
                            start=True, stop=True)
                        rb = avsb_pool.tile([O, 512], F32, tag=f"rb{h % 2}",
                                            name=f"rb{h % 2}")
                        nc.vector.reciprocal_approx_fast(
                            out=rb[:], in_=ps_bc[0:O, :])
                        nc.gpsimd.tensor_tensor(
                            out=outT[base:base + O, kt, bass.ts(nch, 512)],
                            in0=av_sb[0:O, :], in1=rb[:], op=AX.mult)

                for p in range(NPAIR + 1):
                    if p < NPAIR:
                        heads = (2 * p, 2 * p + 1)
                        ydram_all[p] = {
                            h: dram_pool.tile([T, T + 1], FP16,
                                              tag=f"y{h % 2}",
                                              name=f"y{h % 2}")
                            for h in heads}
                        s_all[p] = {
                            h: s_pool.tile([P, NT, T], BF16,
                                           tag=f"s{h % 2}", name=f"s{h % 2}")
                            for h in heads}
                        et_all[p] = {
                            h: et_pool.tile([P, NT, T], FP8,
                                            tag=f"et{h % 2}",
                                            name=f"et{h % 2}")
                            for h in heads}
                    for i in range(NT):
                        if 1 <= p <= NPAIR:
                            emit_acs_nt(p - 1, i)
                        if p < NPAIR:
                            emit_bd_nt(p, i)
                    if 1 <= p <= NPAIR:
                        with nc.named_scope("tx"):
                            for h in (2 * (p - 1), 2 * (p - 1) + 1):
                                for mt in range(NT):
                                    emit_tx(p - 1, h, mt)
                        with nc.named_scope("avf"):
                            for h in (2 * (p - 1), 2 * (p - 1) + 1):
                                for jm in range(NT // 2):
                                    emit_av_mt(p - 1, h, jm)
                                emit_av_fin(p - 1, h)

            # ---- output projection + residual ----
            with (
                tc.tile_pool(name="fin", bufs=4) as fin_pool,
                tc.tile_pool(name="ps_y", bufs=4, space="PSUM") as ps_y_pool,
            ):
                with nc.named_scope("out"):
                    for nt in range(NT):
                        ps_y = ps_y_pool.tile([P, D], F32, tag="ps",
                                              name="ps")
                        for j in range(2):
                            nc.tensor.matmul(
                                ps_y[:],
                                outT[:, 2 * j:2 * j + 2, bass.ts(nt, P)],
                                w_sb["wo"][:, 2 * j:2 * j + 2, :],
                                start=(j == 0), stop=(j == 1),
                                perf_mode=DR)
                        fin = fin_pool.tile([P, D], F32)
                        nc.vector.scalar_tensor_tensor(
                            out=fin[:], in0=ps_y[:], scalar=INV_SCORE,
                            in1=xres_sb[:, nt, :], op0=AX.mult, op1=AX.add)
                        nc.sync.dma_start(out[bass.ts(nt, P), :], fin[:])

    nc.compile()
    return nc


_NC = None


def _get_nc():
    global _NC
    if _NC is None:
        _NC = build_nc()
    return _NC


def _run(inputs_dict, trace=False, trace_cores=None):
    f8 = ml_dtypes.float8_e4m3
    bf = ml_dtypes.bfloat16
    inputs = np.asarray(inputs_dict["inputs"], np.float32)
    pos = np.asarray(inputs_dict["pos"], np.float32)
    gamma = np.asarray(inputs_dict["gamma"], np.float32)
    beta = np.asarray(inputs_dict["beta"], np.float32)
    qk = np.asarray(inputs_dict["query_kernel"], np.float32)
    kk = np.asarray(inputs_dict["key_kernel"], np.float32)
    vk = np.asarray(inputs_dict["value_kernel"], np.float32)
    pk = np.asarray(inputs_dict["pos_kernel"], np.float32)
    u = np.asarray(inputs_dict["pos_bias_u"], np.float32)
    v = np.asarray(inputs_dict["pos_bias_v"], np.float32)
    prk = np.asarray(inputs_dict["projection_kernel"], np.float32)
    pbias = np.asarray(inputs_dict["projection_bias"], np.float32)

    scale = 1.0 / np.sqrt(np.float32(O))

    def wcat(w, rowscale=None):
        c = np.transpose(w, (1, 0, 2)).reshape(D, H * O)
        if rowscale is not None:
            c = c * rowscale[:, None]
        return np.ascontiguousarray(
            c.reshape(KT, P, H * O).transpose(1, 0, 2)).astype(f8)

    wq_c = wcat(qk * SC, gamma)
    wk_c = wcat(kk * SC * scale, gamma)
    wv_c = wcat(vk * SC, gamma)
    wp_c = wcat(pk * SC * scale)
    wo_c = np.ascontiguousarray(
        (prk * SC).reshape(H * O, D).reshape(KT, P, D)
        .transpose(1, 0, 2)).astype(f8)
    u_c = np.ascontiguousarray(
        (u * SC).reshape(H * O).reshape(KT, P).T).astype(np.float32)
    v_c = np.ascontiguousarray(
        (v * SC).reshape(H * O).reshape(KT, P).T).astype(np.float32)
    beta_adj = np.where(gamma != 0, beta / np.where(gamma == 0, 1, gamma), 0.0)
    beta_b = np.broadcast_to(beta_adj[None, :], (P, D)).astype(bf).copy()

    in_maps = []
    for b in range(8):
        x_b = inputs[b]
        in_maps.append({
            "x_res": np.ascontiguousarray(
                x_b.reshape(NT, P, D).transpose(1, 0, 2)).astype(np.float32),
            "post": np.ascontiguousarray(
                pos[b].T.reshape(KT, P, T).transpose(1, 0, 2)).astype(f8),
            "wq": wq_c, "wk": wk_c, "wv": wv_c, "wp": wp_c, "wo": wo_c,
            "u_in": u_c, "v_in": v_c, "beta_in": beta_b,
        })

    nc = _get_nc()
    res = run_bass_kernel_spmd(
        nc, in_maps, core_ids=list(range(8)), trace=trace,
        trace_cores=trace_cores)
    outs = np.stack([np.asarray(r["out"], np.float32) for r in res.results])
    outs = outs + pbias[None, None, :]
    return outs, res


def kernel(**inputs):
    outs, _ = _run(inputs)
    return outs


if __name__ == "__main__":
    nc = build_nc()
    print("built ok")
